# revision 24
# baseline (speedup 1.0000x reference)
"""AiVad (retrieval 1-NN + GMM) Trainium2 kernel — fp8 + LSE-scan version.

Strategy (8 NeuronCores, SPMD; banks sharded 8192 rows/core, queries
replicated):
  - appearance: s[n,m] = 2 q.b via FP8(e4m3) DoubleRow matmul — K=512
    contracts in 2 virtual k-tiles of 256 (2 fp8 weights/PE cell), halving
    tensor-engine time vs bf16. Shard rows are sorted by |b|^2 (host):
    middle 6 groups have near-constant per-lane bias, the 2 tail groups get
    exact per-rank bias on the host. Raw bf16 running maxes (middle) and
    raw tail copies are shipped to HBM; host adds biases and reduces.
    Scan split: ScalarE copies group 1, VectorE max-accumulates groups 2-6
    and copies the tails (tensor ops straight from PSUM).
  - pose: single-bf16 matmul, K=36 rows (34 dims + per-column bias row
    (C - |b|^2) + per-query centering row -c_q). Two bank groups run
    concurrently via PE row-tiling (rows 0-35 / 64-99). The centered PSUM
    scores are reduced by ScalarE in ONE op per group: exp(beta*s) with
    accum_out giving the per-query exp-sum (log-sum-exp ~ max, bias
    ln(n_eff)/beta ~ 5e-3 at beta=6). Host: smax = c_q + ln(sum)/beta.
  - velocity: GMM log-likelihood via fp32 matmul z = (x-mu) @ (L/sqrt(2))
    + ACT Square/Exp/Ln + DVE reduces (exact logsumexp, K=5).
"""

import math

import numpy as np
import ml_dtypes

BF16 = ml_dtypes.bfloat16
F8 = ml_dtypes.float8_e4m3

P = 128          # partitions
N = 2048         # queries
QT = N // P      # 16 query tiles
DA = 512         # appearance dim
DP = 34          # pose dim
DV = 8           # velocity dim
KG = 5           # gmm components
M = 65536        # bank rows
NCORES = 8
MSH = M // NCORES  # 8192 bank rows per core
KT8 = 2          # fp8 DoubleRow virtual k-tiles (2*256 = 512)
G = 8            # m-groups per core
GW = MSH // G    # 1024 group width (2 psum banks)
NCHUNK = GW // 512  # 2 matmul chunks per group
KPOSE = DP + 2   # pose rows: 34 dims + bias row + centering row
BETA_POSE = 6.0
LN2PI = float(np.log(2.0 * np.pi))

_cache: dict = {}


def _split_multi_waits(bir_json: bytes) -> bytes:
    """Split instructions with >1 attached sem-waits into single-wait
    EventSemaphore instructions (this walrus build rejects multi-wait
    encodings with 'Too many sync wait commands'). Waits here are
    monotonic sem-ge waits, so sequential waiting is equivalent."""
    import json as _json

    j = _json.loads(bir_json)
    cnt = [0]

    def fix_block(blk):
        out = []
        for inst in blk.get("instructions", []):
            si = inst.get("sync_info")
            waits = (si or {}).get("on_wait") or []
            if len(waits) > 1:
                for w in waits[:-1]:
                    cnt[0] += 1
                    out.append(
                        {
                            "debug": inst.get("debug", 0),
                            "engine": inst["engine"],
                            "ins": [],
                            "outs": [],
                            "name": f"swait{cnt[0]}_{inst['name']}",
                            "opcode": "EventSemaphore",
                            "sync_info": {"on_update": [], "on_wait": [w]},
                        }
                    )
                si["on_wait"] = [waits[-1]]
            out.append(inst)
        blk["instructions"] = out
        for sb in blk.get("blocks", []):
            fix_block(sb)

    for fn in j["functions"]:
        for blk in fn.get("blocks", []):
            fix_block(blk)
    return _json.dumps(j).encode()


def _install_wait_split_patch():
    import concourse.bass_utils as bu
    import concourse.bass2jax as bj

    if getattr(bu, "_wait_split_patched", False):
        return
    orig = bu.compile_bir_kernel

    def patched(bir_json, tmpdir, neff_name="file.neff"):
        return orig(_split_multi_waits(bytes(bir_json)), tmpdir, neff_name=neff_name)

    bu.compile_bir_kernel = patched
    bj.compile_bir_kernel = patched
    bu._wait_split_patched = True


def _build_bass():
    import concourse.bass as bass
    import concourse.mybir as mybir
    import concourse.tile as tile
    from contextlib import ExitStack

    dt = mybir.dt
    AX = mybir.AxisListType
    ALU = mybir.AluOpType
    AF = mybir.ActivationFunctionType
    DR = mybir.MatmulPerfMode.DoubleRow

    nc = bass.Bass()

    qkt8_d = nc.declare_dram_parameter("qkt8", [P, KT8 * 2 * N], dt.float8e4, isOutput=False)
    bkt8_d = nc.declare_dram_parameter("bkt8", [P, G * KT8 * 2 * GW], dt.float8e4, isOutput=False)
    qp_d = nc.declare_dram_parameter("qkt_pose", [KPOSE, N], dt.bfloat16, isOutput=False)
    bp_d = nc.declare_dram_parameter("bkt_pose", [KPOSE, MSH], dt.bfloat16, isOutput=False)
    gmm_lhs = nc.declare_dram_parameter("gmm_lhs", [DV + 1, N], dt.float32, isOutput=False)
    gmm_rhs = nc.declare_dram_parameter("gmm_rhs", [DV + 1, KG * DV], dt.float32, isOutput=False)
    gmm_c = nc.declare_dram_parameter("gmm_c", [P, QT * KG], dt.float32, isOutput=False)
    out_vel = nc.declare_dram_parameter("out_vel", [P, QT], dt.float32, isOutput=True)
    out_esum = nc.declare_dram_parameter(
        "out_esum", [P, G * QT], dt.float32, isOutput=True
    )
    runs_mid = nc.declare_dram_parameter("runs_mid", [QT, P, GW], dt.bfloat16, isOutput=True)
    runs_t0 = nc.declare_dram_parameter("runs_t0", [QT, P, GW], dt.bfloat16, isOutput=True)
    runs_t1 = nc.declare_dram_parameter("runs_t1", [QT, P, GW], dt.bfloat16, isOutput=True)

    with tile.TileContext(nc) as tc, ExitStack() as ctx:
        const = ctx.enter_context(tc.tile_pool(name="const", bufs=1))
        work = ctx.enter_context(tc.tile_pool(name="work", bufs=4))
        psum = ctx.enter_context(tc.tile_pool(name="psum", bufs=1, space="PSUM"))

        # --- tiles; DMAs in first-use order so PE starts early ---
        qa8 = const.tile([P, KT8 * 2 * N], dt.float8e4, tag="qa8")
        nc.sync.dma_start(qa8[:], qkt8_d[:, :])
        bkt8_r = bkt8_d[:, :].rearrange("p (g x) -> p g x", g=G)
        sb_ba = {}

        # group 0 bank + pose tiles early
        ba0 = const.tile([P, KT8 * 2 * GW], dt.float8e4, tag="ba0")
        nc.sync.dma_start(ba0[:], bkt8_r[:, 0, :])
        sb_ba[0] = ba0
        qp = const.tile([P, N], dt.bfloat16, tag="qp")
        nc.sync.dma_start(qp[0:KPOSE, :], qp_d[:, :])
        nc.sync.dma_start(qp[64:64 + KPOSE, :], qp_d[:, :])
        sb_bp = {}
        for jj in range(G // 2):
            t = const.tile([P, GW], dt.bfloat16, tag=f"bp{jj}", name=f"bp{jj}")
            nc.sync.dma_start(t[0:KPOSE, :], bp_d[:, 2 * jj * GW:(2 * jj + 1) * GW])
            nc.sync.dma_start(
                t[64:64 + KPOSE, :], bp_d[:, (2 * jj + 1) * GW:(2 * jj + 2) * GW]
            )
            sb_bp[jj] = t
            if jj == 0:
                for g in (1, 2):
                    t2 = const.tile([P, KT8 * 2 * GW], dt.float8e4, tag=f"ba{g}", name=f"ba{g}")
                    nc.sync.dma_start(t2[:], bkt8_r[:, g, :])
                    sb_ba[g] = t2
        sb_glhs = const.tile([DV + 1, N], dt.float32, tag="glhs")
        nc.sync.dma_start(sb_glhs[:], gmm_lhs[:, :])
        sb_grhs = const.tile([DV + 1, KG * DV], dt.float32, tag="grhs")
        nc.sync.dma_start(sb_grhs[:], gmm_rhs[:, :])
        sb_gc = const.tile([P, QT * KG], dt.float32, tag="gc")
        nc.sync.dma_start(sb_gc[:], gmm_c[:, :])
        for g in range(3, G):
            t2 = const.tile([P, KT8 * 2 * GW], dt.float8e4, tag=f"ba{g}", name=f"ba{g}")
            nc.sync.dma_start(t2[:], bkt8_r[:, g, :])
            sb_ba[g] = t2

        outp = const.tile([P, QT], dt.float32, tag="outp")
        esums = const.tile([P, G * QT], dt.float32, tag="esums")
        run_a = [
            const.tile([P, GW], dt.bfloat16, tag=f"run{q}", name=f"run{q}")
            for q in range(QT)
        ]

        qa8_r = qa8[:].rearrange("p (t i n) -> p t i n", t=KT8, i=2)

        # --- velocity GMM pieces, interleaved into the main loop ---
        z2 = const.tile([P, QT * KG * DV], dt.float32, tag="z2")

        def gmm_mm(q):
            qs = slice(q * P, (q + 1) * P)
            pg = psum.tile([P, GW], dt.float32, tag="pa", bufs=2, name="pg")
            nc.tensor.matmul(
                pg[:, : KG * DV],
                lhsT=sb_glhs[:, qs],
                rhs=sb_grhs[:],
                start=True,
                stop=True,
            )
            nc.scalar.activation(
                out=z2[:, q * KG * DV:(q + 1) * KG * DV],
                in_=pg[:, : KG * DV],
                func=AF.Square,
            )

        def gmm_chain():
            maha2 = const.tile([P, QT * KG], dt.float32, tag="maha2")
            nc.vector.tensor_reduce(
                out=maha2[:],
                in_=z2[:].rearrange("p (x e) -> p x e", e=DV),
                axis=AX.X,
                op=ALU.add,
            )
            targ = const.tile([P, QT * KG], dt.float32, tag="targ")
            nc.vector.tensor_tensor(targ[:], sb_gc[:], maha2[:], ALU.subtract)
            earg = const.tile([P, QT * KG], dt.float32, tag="earg")
            nc.scalar.activation(out=earg[:], in_=targ[:], func=AF.Exp)
            ssum = const.tile([P, QT], dt.float32, tag="ssum")
            nc.vector.tensor_reduce(
                out=ssum[:],
                in_=earg[:].rearrange("p (t k) -> p t k", k=KG),
                axis=AX.X,
                op=ALU.add,
            )
            nc.scalar.activation(out=outp[:, 0:QT], in_=ssum[:], func=AF.Ln)

        # PE warm-up: matmuls on memset data start within ~1us of kernel
        # launch (no DMA dependency), so the HAM clock gate opens during the
        # input-DMA window instead of the first real phases running at half
        # clock.
        warm = const.tile([P, 512], dt.bfloat16, tag="warm")
        nc.any.memset(warm[:], 1)
        for w in range(12):
            pwu = psum.tile([P, GW], dt.float32, tag="pa", bufs=2, name="pwu")
            nc.tensor.matmul(
                pwu[:, 0:512],
                lhsT=warm[:, 0:P],
                rhs=warm[:, :],
                start=True,
                stop=True,
            )

        # --- main loop --- (phase order: init, maxes, tails mid-kernel
        # once input DMAs have drained, remaining maxes, final ship).
        # Unrolled by 2: both iterations' app matmuls plus the pose pair
        # issue as one contiguous PE burst, then the scans — denser PE
        # activity keeps the HAM clock gate open through ring handoffs.
        for phase, g in enumerate([1, 2, 3, 0, 7, 4, 5, 6]):
            ba_r = sb_ba[g][:].rearrange("p (t i c) -> p t i c", t=KT8, i=2)
            for qq in range(0, QT, 2):
              pas = {}
              for q in (qq, qq + 1):
                qs = slice(q * P, (q + 1) * P)
                # appearance fp8 DoubleRow: 2 virtual k-tiles x 2 chunks
                pa = psum.tile([P, GW], dt.float32, tag="pa", bufs=2, name="pa")
                pas[q] = pa
                for t in range(KT8):
                    for c in range(NCHUNK):
                        cs = slice(c * 512, (c + 1) * 512)
                        nc.tensor.matmul(
                            pa[:, cs],
                            lhsT=qa8_r[:, t, :, qs],
                            rhs=ba_r[:, t, :, cs],
                            start=(t == 0),
                            stop=(t == KT8 - 1),
                            perf_mode=DR,
                        )
              for q in (qq, qq + 1):
                it = phase * QT + q
                qs = slice(q * P, (q + 1) * P)
                pa = pas[q]
                # pose: one group-pair per TWO global iterations, decoupled
                # from the app phase so ScalarE sees a uniform one-LSE-per-
                # 2-iterations load everywhere (no per-phase clustering).
                # Pair p = it//2 handles bank pair p//QT for query tile p%QT
                # via PE row tiling; both groups land in one [128, 2048]
                # psum tile, reduced by a single fused exp-sum ACTIVATE.
                if it % 2 == 1:
                    pidx = it // 2
                    jj, qp_i = divmod(pidx, QT)
                    qsp = slice(qp_i * P, (qp_i + 1) * P)
                    # two independent single-group rings (2 banks each):
                    # each ring's exp-sum has a full block of slack before
                    # its buffer is needed again, so the pose stream is
                    # throughput-bound, not MM->LSE->MM latency-bound.
                    ppA = psum.tile([P, GW], dt.float32, tag="ppA", name="ppA")
                    ppB = psum.tile([P, GW], dt.float32, tag="ppB", name="ppB")
                    bp = sb_bp[jj]
                    for c in range(NCHUNK):
                        cs = slice(c * 512, (c + 1) * 512)
                        nc.tensor.matmul(
                            ppA[:, cs],
                            lhsT=qp[0:KPOSE, qsp],
                            rhs=bp[0:KPOSE, cs],
                            start=True,
                            stop=True,
                        )
                        nc.tensor.matmul(
                            ppB[:, cs],
                            lhsT=qp[64:64 + KPOSE, qsp],
                            rhs=bp[64:64 + KPOSE, cs],
                            start=True,
                            stop=True,
                        )
                    for half, pph in ((0, ppA), (1, ppB)):
                        scr = work.tile([P, GW], dt.bfloat16, tag="scr", name="scr")
                        col = (2 * jj + half) * QT + qp_i
                        nc.scalar.activation(
                            out=scr[:],
                            in_=pph[:],
                            func=AF.Exp,
                            scale=BETA_POSE,
                            accum_out=esums[:, col:col + 1],
                        )

                # appearance scan: tails + most mids on DVE, half of the
                # g==1 init copies on ACT to balance engine load
                if g == 0 or g == G - 1:
                    wt = work.tile([P, GW], dt.bfloat16, tag="wt", name="wt")
                    nc.vector.tensor_copy(wt[:], pa[:])
                    nc.scalar.dma_start(runs_t0[q] if g == 0 else runs_t1[q], wt[:])
                elif g == 1:
                    nc.vector.tensor_copy(run_a[q][:], pa[:])
                else:
                    nc.vector.tensor_tensor(
                        run_a[q][:], run_a[q][:], pa[:], ALU.max
                    )
                    if g == G - 2:
                        nc.scalar.dma_start(runs_mid[q], run_a[q][:])

                if g == 3:
                    gmm_mm(q)
                if g == 4 and q == 0:
                    gmm_chain()

        nc.scalar.dma_start(out_vel[:, :], outp[:])
        nc.scalar.dma_start(out_esum[:, :], esums[:])

    # Hoist matmul sem-waits onto the paired ldweights so the wait overlaps
    # the previous matmul's drain (same pass Bacc.compile runs).
    import bass_rust as _br

    _br.move_matmul_waits_to_ldweights(nc.m)
    return nc


def _get_nc():
    if "nc" not in _cache:
        _cache["nc"] = _build_bass()
    return _cache["nc"]


def prepare(inputs):
    """Host-side shard + layout prep. Returns (in_maps, host_ctx)."""
    velocity = np.asarray(inputs["velocity"], np.float32)
    pose = np.asarray(inputs["pose"], np.float32)
    appearance = np.asarray(inputs["appearance"], np.float32)
    pose_bank = np.asarray(inputs["pose_bank"], np.float32)
    feature_bank = np.asarray(inputs["feature_bank"], np.float32)
    gmm_means = np.asarray(inputs["gmm_means"], np.float64)
    gmm_prec_chol = np.asarray(inputs["gmm_prec_chol"], np.float64)
    gmm_log_weights = np.asarray(inputs["gmm_log_weights"], np.float64)

    # ---- appearance queries: fp8, DoubleRow layout [p, (t i n)] ----
    A2 = (2.0 * appearance).astype(F8)  # [2048, 512]
    qkt8 = np.ascontiguousarray(
        A2.T.reshape(KT8, 2, P, N).transpose(2, 0, 1, 3).reshape(P, KT8 * 2 * N)
    )

    # ---- pose queries: [36, 2048] bf16 ----
    q2_pose = (pose.astype(np.float64) ** 2).sum(1)
    c_pose = 2.0 * np.sqrt(q2_pose) * math.sqrt(2.0 * math.log(M))
    c_pose_bf = c_pose.astype(np.float32).astype(BF16)
    qkt_pose = np.empty((KPOSE, N), BF16)
    qkt_pose[0:DP] = (2.0 * pose).T.astype(BF16)
    qkt_pose[DP] = np.ones(N, BF16)
    qkt_pose[DP + 1] = -c_pose_bf

    # ---- gmm constants (identical to the exact baseline) ----
    pcs = gmm_prec_chol / math.sqrt(2.0)  # [5, 8, 8]
    gmm_rhs = np.empty((DV + 1, KG * DV), np.float32)
    for k in range(KG):
        gmm_rhs[0:DV, k * DV:(k + 1) * DV] = pcs[k]
        gmm_rhs[DV, k * DV:(k + 1) * DV] = -(gmm_means[k] @ pcs[k])
    logdet = np.log(np.diagonal(gmm_prec_chol, axis1=1, axis2=2)).sum(1)  # [5]
    c5 = gmm_log_weights + logdet - 0.5 * DV * LN2PI
    gmm_c = np.broadcast_to(
        np.tile(c5.astype(np.float32), QT), (P, QT * KG)
    ).copy()
    gmm_lhs = np.empty((DV + 1, N), np.float32)
    gmm_lhs[0:DV] = velocity.T
    gmm_lhs[DV] = 1.0

    # ---- bank norms / constants (global, f64) ----
    b2_app = (feature_bank.astype(np.float64) ** 2).sum(1)  # [65536]
    b2_pose = (pose_bank.astype(np.float64) ** 2).sum(1)
    C_app = float(b2_app.mean())
    C_pose = float(b2_pose.mean())
    q2_app = (appearance.astype(np.float64) ** 2).sum(1)  # [2048]

    in_maps = []
    cls = []
    cv0s = []
    cv1s = []
    for ci in range(NCORES):
        sl = slice(ci * MSH, (ci + 1) * MSH)
        B = feature_bank[sl]  # [8192, 512]
        b2s = b2_app[sl]
        # Sort rows by |b|^2: tail groups 0 / G-1 take the extreme GW ranks
        # (host applies exact per-rank bias); the middle 6 groups interleave
        # so each lane-column sees 6 consecutive mid ranks (near-constant
        # bias, host adds the per-column mean).
        order = np.argsort(b2s, kind="stable")  # rank -> row
        nmid = G - 2
        rank_of_col = np.empty((G, GW), np.int64)
        rank_of_col[0] = np.arange(GW)
        rank_of_col[G - 1] = MSH - GW + np.arange(GW)
        for gg in range(1, G - 1):
            rank_of_col[gg] = GW + np.arange(GW) * nmid + (gg - 1)
        cols = order[rank_of_col.reshape(MSH)]
        B8 = B[cols].astype(F8)  # [8192, 512] fp8, sorted layout
        # layout [p, (g t i c)]
        bkt8 = np.ascontiguousarray(
            B8.T.reshape(KT8, 2, P, G, GW).transpose(2, 3, 0, 1, 4).reshape(
                P, G * KT8 * 2 * GW
            )
        )
        cval = C_app - b2s[order]  # [8192] by rank
        cls.append(cval[GW:MSH - GW].reshape(GW, nmid).mean(1))
        cv0s.append(cval[rank_of_col[0]])
        cv1s.append(cval[rank_of_col[G - 1]])

        Bp = pose_bank[sl]  # [8192, 34]
        bkt_pose = np.empty((KPOSE, MSH), BF16)
        bkt_pose[0:DP] = Bp.T.astype(BF16)
        bkt_pose[DP] = (C_pose - b2_pose[sl]).astype(np.float32).astype(BF16)
        bkt_pose[DP + 1] = np.ones(MSH, BF16)

        in_maps.append(
            {
                "qkt8": qkt8,
                "bkt8": bkt8,
                "qkt_pose": qkt_pose,
                "bkt_pose": bkt_pose,
                "gmm_lhs": gmm_lhs,
                "gmm_rhs": gmm_rhs,
                "gmm_c": gmm_c,
            }
        )

    host_ctx = {
        "cls": cls,
        "cv0s": cv0s,
        "cv1s": cv1s,
        "q2_app": q2_app,
        "q2_pose": q2_pose,
        "C_app": C_app,
        "C_pose": C_pose,
        "c_pose_bf": c_pose_bf.astype(np.float64),
        "vel_min": float(np.asarray(inputs["vel_min"]).reshape(-1)[0]),
        "vel_max": float(np.asarray(inputs["vel_max"]).reshape(-1)[0]),
        "pose_min": float(np.asarray(inputs["pose_min"]).reshape(-1)[0]),
        "pose_max": float(np.asarray(inputs["pose_max"]).reshape(-1)[0]),
        "feat_min": float(np.asarray(inputs["feat_min"]).reshape(-1)[0]),
        "feat_max": float(np.asarray(inputs["feat_max"]).reshape(-1)[0]),
    }
    return in_maps, host_ctx


def combine(results, host_ctx):
    """Gather per-core partials -> full [3, 2048] output."""
    # appearance: raw 2q.b maxes; host adds (C - |b|^2) per lane/rank
    smax_app = np.full(N, -np.inf)
    S_pose = np.zeros((P, G * QT), np.float64)
    for ci, r in enumerate(results):
        mid = (np.asarray(r["runs_mid"], np.float64)
               + host_ctx["cls"][ci][None, None, :]).max(-1).reshape(N)
        t0 = (np.asarray(r["runs_t0"], np.float64)
              + host_ctx["cv0s"][ci][None, None, :]).max(-1).reshape(N)
        t1 = (np.asarray(r["runs_t1"], np.float64)
              + host_ctx["cv1s"][ci][None, None, :]).max(-1).reshape(N)
        smax_app = np.maximum.reduce([smax_app, mid, t0, t1])
        S_pose += np.asarray(r["out_esum"], np.float64)

    # pose: combine exp-sums (associative) -> LSE
    S_tot = S_pose.reshape(P, G, QT).sum(1)  # [128, 16]
    S_n = S_tot.T.reshape(N)  # n = q*128 + p
    smax_pose = host_ctx["c_pose_bf"] + np.log(S_n) / BETA_POSE

    loglik = np.asarray(results[0]["out_vel"], np.float64).T.reshape(N)

    d2a = host_ctx["q2_app"] + host_ctx["C_app"] - smax_app
    d2p = host_ctx["q2_pose"] + host_ctx["C_pose"] - smax_pose
    dist_a = np.sqrt(np.maximum(d2a, 1e-12))
    dist_p = np.sqrt(np.maximum(d2p, 1e-12))

    vel_s = (-loglik - host_ctx["vel_min"]) / (host_ctx["vel_max"] - host_ctx["vel_min"])
    pose_s = (dist_p - host_ctx["pose_min"]) / (host_ctx["pose_max"] - host_ctx["pose_min"])
    app_s = (dist_a - host_ctx["feat_min"]) / (host_ctx["feat_max"] - host_ctx["feat_min"])
    return np.stack([vel_s, pose_s, app_s]).astype(np.float32)


def run_device(in_maps, trace=False, **kwargs):
    from concourse.bass_utils import run_bass_kernel_spmd

    _install_wait_split_patch()
    return run_bass_kernel_spmd(
        _get_nc(), in_maps, list(range(NCORES)), trace=trace, **kwargs
    )


def kernel(**inputs) -> np.ndarray:
    in_maps, host_ctx = prepare(inputs)
    res = run_device(in_maps)
    return combine(res.results, host_ctx)


# revision 26
# speedup vs baseline: 1.0881x; 1.0881x over previous
"""AiVad (retrieval 1-NN + GMM) Trainium2 kernel — fp8 + LSE-scan version.

Strategy (8 NeuronCores, SPMD; banks sharded 8192 rows/core, queries
replicated):
  - appearance: s[n,m] = 2 q.b via FP8(e4m3) DoubleRow matmul — K=512
    contracts in 2 virtual k-tiles of 256 (2 fp8 weights/PE cell), halving
    tensor-engine time vs bf16. Shard rows are sorted by |b|^2 (host):
    middle 6 groups have near-constant per-lane bias, the 2 tail groups get
    exact per-rank bias on the host. Raw bf16 running maxes (middle) and
    raw tail copies are shipped to HBM; host adds biases and reduces.
    Scan split: ScalarE copies group 1, VectorE max-accumulates groups 2-6
    and copies the tails (tensor ops straight from PSUM).
  - pose: single-bf16 matmul, K=36 rows (34 dims + per-column bias row
    (C - |b|^2) + per-query centering row -c_q). Two bank groups run
    concurrently via PE row-tiling (rows 0-35 / 64-99). The centered PSUM
    scores are reduced by ScalarE in ONE op per group: exp(beta*s) with
    accum_out giving the per-query exp-sum (log-sum-exp ~ max, bias
    ln(n_eff)/beta ~ 5e-3 at beta=6). Host: smax = c_q + ln(sum)/beta.
  - velocity: GMM log-likelihood via fp32 matmul z = (x-mu) @ (L/sqrt(2))
    + ACT Square/Exp/Ln + DVE reduces (exact logsumexp, K=5).
"""

import math

import numpy as np
import ml_dtypes

BF16 = ml_dtypes.bfloat16
F8 = ml_dtypes.float8_e4m3

P = 128          # partitions
N = 2048         # queries
QT = N // P      # 16 query tiles
DA = 512         # appearance dim
DP = 34          # pose dim
DV = 8           # velocity dim
KG = 5           # gmm components
M = 65536        # bank rows
NCORES = 8
MSH = M // NCORES  # 8192 bank rows per core
KT8 = 2          # fp8 DoubleRow virtual k-tiles (2*256 = 512)
G = 8            # m-groups per core
GW = MSH // G    # 1024 group width (2 psum banks)
NCHUNK = GW // 512  # 2 matmul chunks per group
KPOSE = DP + 2   # pose rows: 34 dims + bias row + centering row
BETA_POSE = 6.0
LN2PI = float(np.log(2.0 * np.pi))

_cache: dict = {}


def _split_multi_waits(bir_json: bytes) -> bytes:
    """Split instructions with >1 attached sem-waits into single-wait
    EventSemaphore instructions (this walrus build rejects multi-wait
    encodings with 'Too many sync wait commands'). Waits here are
    monotonic sem-ge waits, so sequential waiting is equivalent."""
    import json as _json

    j = _json.loads(bir_json)
    cnt = [0]

    def fix_block(blk):
        out = []
        for inst in blk.get("instructions", []):
            si = inst.get("sync_info")
            waits = (si or {}).get("on_wait") or []
            if len(waits) > 1:
                for w in waits[:-1]:
                    cnt[0] += 1
                    out.append(
                        {
                            "debug": inst.get("debug", 0),
                            "engine": inst["engine"],
                            "ins": [],
                            "outs": [],
                            "name": f"swait{cnt[0]}_{inst['name']}",
                            "opcode": "EventSemaphore",
                            "sync_info": {"on_update": [], "on_wait": [w]},
                        }
                    )
                si["on_wait"] = [waits[-1]]
            out.append(inst)
        blk["instructions"] = out
        for sb in blk.get("blocks", []):
            fix_block(sb)

    for fn in j["functions"]:
        for blk in fn.get("blocks", []):
            fix_block(blk)
    return _json.dumps(j).encode()


def _install_wait_split_patch():
    import concourse.bass_utils as bu
    import concourse.bass2jax as bj

    if getattr(bu, "_wait_split_patched", False):
        return
    orig = bu.compile_bir_kernel

    def patched(bir_json, tmpdir, neff_name="file.neff"):
        return orig(_split_multi_waits(bytes(bir_json)), tmpdir, neff_name=neff_name)

    bu.compile_bir_kernel = patched
    bj.compile_bir_kernel = patched
    bu._wait_split_patched = True


def _build_bass():
    import concourse.bass as bass
    import concourse.mybir as mybir
    import concourse.tile as tile
    from contextlib import ExitStack

    dt = mybir.dt
    AX = mybir.AxisListType
    ALU = mybir.AluOpType
    AF = mybir.ActivationFunctionType
    DR = mybir.MatmulPerfMode.DoubleRow

    nc = bass.Bass()

    qkt8_d = nc.declare_dram_parameter("qkt8", [P, KT8 * 2 * N], dt.float8e4, isOutput=False)
    bkt8_d = nc.declare_dram_parameter("bkt8", [P, G * KT8 * 2 * GW], dt.float8e4, isOutput=False)
    qp_d = nc.declare_dram_parameter("qkt_pose", [KPOSE, N], dt.bfloat16, isOutput=False)
    bp_d = nc.declare_dram_parameter("bkt_pose", [KPOSE, MSH], dt.bfloat16, isOutput=False)
    gmm_lhs = nc.declare_dram_parameter("gmm_lhs", [DV + 1, N], dt.float32, isOutput=False)
    gmm_rhs = nc.declare_dram_parameter("gmm_rhs", [DV + 1, KG * DV], dt.float32, isOutput=False)
    gmm_c = nc.declare_dram_parameter("gmm_c", [P, QT * KG], dt.float32, isOutput=False)
    out_vel = nc.declare_dram_parameter("out_vel", [P, QT], dt.float32, isOutput=True)
    out_esum = nc.declare_dram_parameter(
        "out_esum", [P, G * QT], dt.float32, isOutput=True
    )
    runs_mid = nc.declare_dram_parameter("runs_mid", [QT, P, GW], dt.bfloat16, isOutput=True)
    runs_t0 = nc.declare_dram_parameter("runs_t0", [QT, P, GW], dt.bfloat16, isOutput=True)
    runs_t1 = nc.declare_dram_parameter("runs_t1", [QT, P, GW], dt.bfloat16, isOutput=True)

    with tile.TileContext(nc) as tc, ExitStack() as ctx:
        const = ctx.enter_context(tc.tile_pool(name="const", bufs=1))
        work = ctx.enter_context(tc.tile_pool(name="work", bufs=4))
        psum = ctx.enter_context(tc.tile_pool(name="psum", bufs=1, space="PSUM"))

        # --- tiles; DMAs in first-use order so PE starts early ---
        qa8 = const.tile([P, KT8 * 2 * N], dt.float8e4, tag="qa8")
        nc.sync.dma_start(qa8[:], qkt8_d[:, :])
        bkt8_r = bkt8_d[:, :].rearrange("p (g x) -> p g x", g=G)
        sb_ba = {}

        # group 0 bank + pose tiles early
        ba0 = const.tile([P, KT8 * 2 * GW], dt.float8e4, tag="ba0")
        nc.sync.dma_start(ba0[:], bkt8_r[:, 0, :])
        sb_ba[0] = ba0
        qp = const.tile([P, N], dt.bfloat16, tag="qp")
        nc.sync.dma_start(qp[0:KPOSE, :], qp_d[:, :])
        nc.sync.dma_start(qp[64:64 + KPOSE, :], qp_d[:, :])
        sb_bp = {}
        for jj in range(G // 2):
            t = const.tile([P, GW], dt.bfloat16, tag=f"bp{jj}", name=f"bp{jj}")
            nc.sync.dma_start(t[0:KPOSE, :], bp_d[:, 2 * jj * GW:(2 * jj + 1) * GW])
            nc.sync.dma_start(
                t[64:64 + KPOSE, :], bp_d[:, (2 * jj + 1) * GW:(2 * jj + 2) * GW]
            )
            sb_bp[jj] = t
            if jj == 0:
                for g in (1, 2):
                    t2 = const.tile([P, KT8 * 2 * GW], dt.float8e4, tag=f"ba{g}", name=f"ba{g}")
                    nc.sync.dma_start(t2[:], bkt8_r[:, g, :])
                    sb_ba[g] = t2
        sb_glhs = const.tile([DV + 1, N], dt.float32, tag="glhs")
        nc.sync.dma_start(sb_glhs[:], gmm_lhs[:, :])
        sb_grhs = const.tile([DV + 1, KG * DV], dt.float32, tag="grhs")
        nc.sync.dma_start(sb_grhs[:], gmm_rhs[:, :])
        sb_gc = const.tile([P, QT * KG], dt.float32, tag="gc")
        nc.sync.dma_start(sb_gc[:], gmm_c[:, :])
        for g in range(3, G):
            t2 = const.tile([P, KT8 * 2 * GW], dt.float8e4, tag=f"ba{g}", name=f"ba{g}")
            nc.sync.dma_start(t2[:], bkt8_r[:, g, :])
            sb_ba[g] = t2

        outp = const.tile([P, QT], dt.float32, tag="outp")
        esums = const.tile([P, G * QT], dt.float32, tag="esums")
        run_a = [
            const.tile([P, GW], dt.bfloat16, tag=f"run{q}", name=f"run{q}")
            for q in range(QT)
        ]

        qa8_r = qa8[:].rearrange("p (t i n) -> p t i n", t=KT8, i=2)

        # --- velocity GMM pieces, interleaved into the main loop ---
        z2 = const.tile([P, QT * KG * DV], dt.float32, tag="z2")

        def gmm_mm(q):
            qs = slice(q * P, (q + 1) * P)
            pg = psum.tile([P, GW], dt.float32, tag="pa", bufs=2, name="pg")
            nc.tensor.matmul(
                pg[:, : KG * DV],
                lhsT=sb_glhs[:, qs],
                rhs=sb_grhs[:],
                start=True,
                stop=True,
            )
            nc.scalar.activation(
                out=z2[:, q * KG * DV:(q + 1) * KG * DV],
                in_=pg[:, : KG * DV],
                func=AF.Square,
            )

        def gmm_chain():
            maha2 = const.tile([P, QT * KG], dt.float32, tag="maha2")
            nc.vector.tensor_reduce(
                out=maha2[:],
                in_=z2[:].rearrange("p (x e) -> p x e", e=DV),
                axis=AX.X,
                op=ALU.add,
            )
            targ = const.tile([P, QT * KG], dt.float32, tag="targ")
            nc.vector.tensor_tensor(targ[:], sb_gc[:], maha2[:], ALU.subtract)
            earg = const.tile([P, QT * KG], dt.float32, tag="earg")
            nc.scalar.activation(out=earg[:], in_=targ[:], func=AF.Exp)
            ssum = const.tile([P, QT], dt.float32, tag="ssum")
            nc.vector.tensor_reduce(
                out=ssum[:],
                in_=earg[:].rearrange("p (t k) -> p t k", k=KG),
                axis=AX.X,
                op=ALU.add,
            )
            nc.scalar.activation(out=outp[:, 0:QT], in_=ssum[:], func=AF.Ln)

        # PE warm-up: matmuls on memset data start within ~1us of kernel
        # launch (no DMA dependency), so the HAM clock gate opens during the
        # input-DMA window instead of the first real phases running at half
        # clock.
        warm = const.tile([P, 512], dt.bfloat16, tag="warm")
        nc.any.memset(warm[:], 1)
        for w in range(12):
            pwu = psum.tile([P, GW], dt.float32, tag="pa", bufs=2, name="pwu")
            nc.tensor.matmul(
                pwu[:, 0:512],
                lhsT=warm[:, 0:P],
                rhs=warm[:, :],
                start=True,
                stop=True,
            )

        # --- main loop --- (phase order: init, maxes, tails mid-kernel
        # once input DMAs have drained, remaining maxes, final ship).
        # Unrolled by 2: both iterations' app matmuls plus the pose pair
        # issue as one contiguous PE burst, then the scans — denser PE
        # activity keeps the HAM clock gate open through ring handoffs.
        for phase, g in enumerate([1, 2, 3, 0, 7, 4, 5, 6]):
            ba_r = sb_ba[g][:].rearrange("p (t i c) -> p t i c", t=KT8, i=2)
            for qq in range(0, QT, 2):
              pas = {}
              for q in (qq, qq + 1):
                qs = slice(q * P, (q + 1) * P)
                # appearance fp8 DoubleRow: 2 virtual k-tiles x 2 chunks
                pa = psum.tile([P, GW], dt.float32, tag="pa", bufs=2, name="pa")
                pas[q] = pa
                for t in range(KT8):
                    for c in range(NCHUNK):
                        cs = slice(c * 512, (c + 1) * 512)
                        nc.tensor.matmul(
                            pa[:, cs],
                            lhsT=qa8_r[:, t, :, qs],
                            rhs=ba_r[:, t, :, cs],
                            start=(t == 0),
                            stop=(t == KT8 - 1),
                            perf_mode=DR,
                        )
              for q in (qq, qq + 1):
                it = phase * QT + q
                qs = slice(q * P, (q + 1) * P)
                pa = pas[q]
                # pose: one group-pair per TWO global iterations, decoupled
                # from the app phase so ScalarE sees a uniform one-LSE-per-
                # 2-iterations load everywhere (no per-phase clustering).
                # Pair p = it//2 handles bank pair p//QT for query tile p%QT
                # via PE row tiling; both groups land in one [128, 2048]
                # psum tile, reduced by a single fused exp-sum ACTIVATE.
                if it % 2 == 1:
                    pidx = it // 2
                    jj, qp_i = divmod(pidx, QT)
                    qsp = slice(qp_i * P, (qp_i + 1) * P)
                    # two independent single-group rings (2 banks each):
                    # each ring's exp-sum has a full block of slack before
                    # its buffer is needed again, so the pose stream is
                    # throughput-bound, not MM->LSE->MM latency-bound.
                    ppA = psum.tile([P, GW], dt.float32, tag="ppA", name="ppA")
                    ppB = psum.tile([P, GW], dt.float32, tag="ppB", name="ppB")
                    bp = sb_bp[jj]
                    for c in range(NCHUNK):
                        cs = slice(c * 512, (c + 1) * 512)
                        nc.tensor.matmul(
                            ppA[:, cs],
                            lhsT=qp[0:KPOSE, qsp],
                            rhs=bp[0:KPOSE, cs],
                            start=True,
                            stop=True,
                        )
                        nc.tensor.matmul(
                            ppB[:, cs],
                            lhsT=qp[64:64 + KPOSE, qsp],
                            rhs=bp[64:64 + KPOSE, cs],
                            start=True,
                            stop=True,
                        )
                    for half, pph in ((0, ppA), (1, ppB)):
                        scr = work.tile([P, GW], dt.bfloat16, tag="scr", name="scr")
                        col = (2 * jj + half) * QT + qp_i
                        nc.scalar.activation(
                            out=scr[:],
                            in_=pph[:],
                            func=AF.Exp,
                            scale=BETA_POSE,
                            accum_out=esums[:, col:col + 1],
                        )

                # appearance scan: tails + most mids on DVE, half of the
                # g==1 init copies on ACT to balance engine load
                if g == 0 or g == G - 1:
                    wt = work.tile([P, GW], dt.bfloat16, tag="wt", name="wt")
                    nc.vector.tensor_copy(wt[:], pa[:])
                    nc.scalar.dma_start(runs_t0[q] if g == 0 else runs_t1[q], wt[:])
                elif g == 1:
                    nc.vector.tensor_copy(run_a[q][:], pa[:])
                else:
                    nc.vector.tensor_tensor(
                        run_a[q][:], run_a[q][:], pa[:], ALU.max
                    )
                    if g == G - 2:
                        nc.scalar.dma_start(runs_mid[q], run_a[q][:])

                if g == 3:
                    gmm_mm(q)
                if g == 4 and q == 0:
                    gmm_chain()

        nc.scalar.dma_start(out_vel[:, :], outp[:])
        nc.scalar.dma_start(out_esum[:, :], esums[:])

    # Hoist matmul sem-waits onto the paired ldweights so the wait overlaps
    # the previous matmul's drain (same pass Bacc.compile runs).
    import bass_rust as _br

    _br.move_matmul_waits_to_ldweights(nc.m)
    return nc


def _get_nc():
    if "nc" not in _cache:
        _cache["nc"] = _build_bass()
    return _cache["nc"]


def prepare(inputs):
    """Host-side shard + layout prep. Returns (in_maps, host_ctx)."""
    velocity = np.asarray(inputs["velocity"], np.float32)
    pose = np.asarray(inputs["pose"], np.float32)
    appearance = np.asarray(inputs["appearance"], np.float32)
    pose_bank = np.asarray(inputs["pose_bank"], np.float32)
    feature_bank = np.asarray(inputs["feature_bank"], np.float32)
    gmm_means = np.asarray(inputs["gmm_means"], np.float64)
    gmm_prec_chol = np.asarray(inputs["gmm_prec_chol"], np.float64)
    gmm_log_weights = np.asarray(inputs["gmm_log_weights"], np.float64)

    # ---- appearance queries: fp8, DoubleRow layout [p, (t i n)] ----
    A2 = (2.0 * appearance).astype(F8)  # [2048, 512]
    qkt8 = np.ascontiguousarray(
        A2.T.reshape(KT8, 2, P, N).transpose(2, 0, 1, 3).reshape(P, KT8 * 2 * N)
    )

    # ---- pose queries: [36, 2048] bf16 ----
    q2_pose = (pose.astype(np.float64) ** 2).sum(1)
    c_pose = 2.0 * np.sqrt(q2_pose) * math.sqrt(2.0 * math.log(M))
    c_pose_bf = c_pose.astype(np.float32).astype(BF16)
    qkt_pose = np.empty((KPOSE, N), BF16)
    qkt_pose[0:DP] = (2.0 * pose).T.astype(BF16)
    qkt_pose[DP] = np.ones(N, BF16)
    qkt_pose[DP + 1] = -c_pose_bf

    # ---- gmm constants (identical to the exact baseline) ----
    pcs = gmm_prec_chol / math.sqrt(2.0)  # [5, 8, 8]
    gmm_rhs = np.empty((DV + 1, KG * DV), np.float32)
    for k in range(KG):
        gmm_rhs[0:DV, k * DV:(k + 1) * DV] = pcs[k]
        gmm_rhs[DV, k * DV:(k + 1) * DV] = -(gmm_means[k] @ pcs[k])
    logdet = np.log(np.diagonal(gmm_prec_chol, axis1=1, axis2=2)).sum(1)  # [5]
    c5 = gmm_log_weights + logdet - 0.5 * DV * LN2PI
    gmm_c = np.broadcast_to(
        np.tile(c5.astype(np.float32), QT), (P, QT * KG)
    ).copy()
    gmm_lhs = np.empty((DV + 1, N), np.float32)
    gmm_lhs[0:DV] = velocity.T
    gmm_lhs[DV] = 1.0

    # ---- bank norms / constants (global, f64) ----
    b2_app = (feature_bank.astype(np.float64) ** 2).sum(1)  # [65536]
    b2_pose = (pose_bank.astype(np.float64) ** 2).sum(1)
    C_app = float(b2_app.mean())
    C_pose = float(b2_pose.mean())
    q2_app = (appearance.astype(np.float64) ** 2).sum(1)  # [2048]

    in_maps = []
    cls = []
    cv0s = []
    cv1s = []
    for ci in range(NCORES):
        sl = slice(ci * MSH, (ci + 1) * MSH)
        B = feature_bank[sl]  # [8192, 512]
        b2s = b2_app[sl]
        # Sort rows by |b|^2: tail groups 0 / G-1 take the extreme GW ranks
        # (host applies exact per-rank bias); the middle 6 groups interleave
        # so each lane-column sees 6 consecutive mid ranks (near-constant
        # bias, host adds the per-column mean).
        order = np.argsort(b2s, kind="stable")  # rank -> row
        nmid = G - 2
        rank_of_col = np.empty((G, GW), np.int64)
        rank_of_col[0] = np.arange(GW)
        rank_of_col[G - 1] = MSH - GW + np.arange(GW)
        for gg in range(1, G - 1):
            rank_of_col[gg] = GW + np.arange(GW) * nmid + (gg - 1)
        cols = order[rank_of_col.reshape(MSH)]
        B8 = B[cols].astype(F8)  # [8192, 512] fp8, sorted layout
        # layout [p, (g t i c)]
        bkt8 = np.ascontiguousarray(
            B8.T.reshape(KT8, 2, P, G, GW).transpose(2, 3, 0, 1, 4).reshape(
                P, G * KT8 * 2 * GW
            )
        )
        cval = C_app - b2s[order]  # [8192] by rank
        cls.append(cval[GW:MSH - GW].reshape(GW, nmid).mean(1))
        cv0s.append(cval[rank_of_col[0]])
        cv1s.append(cval[rank_of_col[G - 1]])

        Bp = pose_bank[sl]  # [8192, 34]
        bkt_pose = np.empty((KPOSE, MSH), BF16)
        bkt_pose[0:DP] = Bp.T.astype(BF16)
        bkt_pose[DP] = (C_pose - b2_pose[sl]).astype(np.float32).astype(BF16)
        bkt_pose[DP + 1] = np.ones(MSH, BF16)

        in_maps.append(
            {
                "qkt8": qkt8,
                "bkt8": bkt8,
                "qkt_pose": qkt_pose,
                "bkt_pose": bkt_pose,
                "gmm_lhs": gmm_lhs,
                "gmm_rhs": gmm_rhs,
                "gmm_c": gmm_c,
            }
        )

    host_ctx = {
        "cls": cls,
        "cv0s": cv0s,
        "cv1s": cv1s,
        "q2_app": q2_app,
        "q2_pose": q2_pose,
        "C_app": C_app,
        "C_pose": C_pose,
        "c_pose_bf": c_pose_bf.astype(np.float64),
        "vel_min": float(np.asarray(inputs["vel_min"]).reshape(-1)[0]),
        "vel_max": float(np.asarray(inputs["vel_max"]).reshape(-1)[0]),
        "pose_min": float(np.asarray(inputs["pose_min"]).reshape(-1)[0]),
        "pose_max": float(np.asarray(inputs["pose_max"]).reshape(-1)[0]),
        "feat_min": float(np.asarray(inputs["feat_min"]).reshape(-1)[0]),
        "feat_max": float(np.asarray(inputs["feat_max"]).reshape(-1)[0]),
    }
    return in_maps, host_ctx


def combine(results, host_ctx):
    """Gather per-core partials -> full [3, 2048] output."""
    # appearance: raw 2q.b maxes; host adds (C - |b|^2) per lane/rank
    smax_app = np.full(N, -np.inf)
    S_pose = np.zeros((P, G * QT), np.float64)
    for ci, r in enumerate(results):
        mid = (np.asarray(r["runs_mid"], np.float64)
               + host_ctx["cls"][ci][None, None, :]).max(-1).reshape(N)
        t0 = (np.asarray(r["runs_t0"], np.float64)
              + host_ctx["cv0s"][ci][None, None, :]).max(-1).reshape(N)
        t1 = (np.asarray(r["runs_t1"], np.float64)
              + host_ctx["cv1s"][ci][None, None, :]).max(-1).reshape(N)
        smax_app = np.maximum.reduce([smax_app, mid, t0, t1])
        S_pose += np.asarray(r["out_esum"], np.float64)

    # pose: combine exp-sums (associative) -> LSE
    S_tot = S_pose.reshape(P, G, QT).sum(1)  # [128, 16]
    S_n = S_tot.T.reshape(N)  # n = q*128 + p
    smax_pose = host_ctx["c_pose_bf"] + np.log(S_n) / BETA_POSE

    loglik = np.asarray(results[0]["out_vel"], np.float64).T.reshape(N)

    d2a = host_ctx["q2_app"] + host_ctx["C_app"] - smax_app
    d2p = host_ctx["q2_pose"] + host_ctx["C_pose"] - smax_pose
    dist_a = np.sqrt(np.maximum(d2a, 1e-12))
    dist_p = np.sqrt(np.maximum(d2p, 1e-12))

    vel_s = (-loglik - host_ctx["vel_min"]) / (host_ctx["vel_max"] - host_ctx["vel_min"])
    pose_s = (dist_p - host_ctx["pose_min"]) / (host_ctx["pose_max"] - host_ctx["pose_min"])
    app_s = (dist_a - host_ctx["feat_min"]) / (host_ctx["feat_max"] - host_ctx["feat_min"])
    return np.stack([vel_s, pose_s, app_s]).astype(np.float32)


def run_device(in_maps, trace=False, **kwargs):
    from concourse.bass_utils import run_bass_kernel_spmd

    _install_wait_split_patch()
    return run_bass_kernel_spmd(
        _get_nc(), in_maps, list(range(NCORES)), trace=trace, **kwargs
    )


def kernel(**inputs) -> np.ndarray:
    in_maps, host_ctx = prepare(inputs)
    res = run_device(in_maps)
    return combine(res.results, host_ctx)
